# revision 44
# baseline (speedup 1.0000x reference)
"""EvidenceNet pairwise-MLP scoring kernel for 8 Trainium2 NeuronCores.

Math (reference):
    img = sign(images_hash)/8, txt = sign(texts_hash)/8          [1024, 64] each
    a[i,k] = (img @ W1[:, :64].T)[i,k] + b1[k]                   [1024, 128]
    t[j,k] = (txt @ W1[:, 64:].T)[j,k]                           [1024, 128]
    negE[i,j] = sum_k W2[0,k] * relu(a[i,k] + t[j,k]) + b2[0]
    posE[i,j] = img[i,:] @ txt[j,:]
    out = [exp(posE/0.5), exp(negE/0.5)] flattened               [1024*1024, 2]
    (clip at +-15 never binds)

Distribution: data-parallel over image rows; core c owns i in [128c, 128c+128).

Sign handling: VectorE computes m2 = (x>=0)*2 in {2,0} (is_ge/mult tensor_scalar,
4x bf16) so nothing on the critical path waits for ScalarE's ~2.7us activation
table load.  sign = m2 - 1; the -1 is folded host-side into the aT bias
(b1 - 0.125*W1.sum(axis=1)) for the neg path, and sgn tiles are derived on
VectorE for the pos path.

Per-core device program (k = the 128 hidden dims lives on partitions):
    tT_h [128k, 1024j]  = W1_txt^T-matmul of m2(txt)   (f32 PSUM, bf16 SBUF copy)
    aT   [128k, 128i]   = W1_img^T-matmul of m2(img) + b1_eff   (f32, SBUF)
    per i (~95 rows on VectorE 4x bf16 tensor_scalar, ~33 on ScalarE from PSUM):
        r_i [128k, 1024j] = relu(tT_h + aT[:, i])      (bf16)
        for jb in 0..8:
            psum[i//32][:, 8*(i%32)+jb] = matmul(lhsT=r_i[:, 128jb:+128], rhs=w2col)
    psum tile t holds negE[i, j] at [j%128, 8*(i%32) + j//128] for i in 32t..32t+32
    evict per tile: ACT exp(2x+2b2) -> negsb[:, 256t:+256]; contiguous dma out.
    out_pos = exp(posE/32), posE = sgn-img x sgn-txt matmul (exact bf16)
Host gathers: col0 = pos rows, col1 = negO.reshape(128,128,8).transpose(1,2,0).
"""
import numpy as np
import ml_dtypes

N_CORES = 8
NI, NT, D, H = 1024, 1024, 64, 128
NI_LOC = NI // N_CORES  # 128
NJB = NT // H           # 8 column blocks
R_BUFS = 40             # in-flight relu tiles (DVE/ACT run-ahead over PE)

_compiled = None


ACT_NUM, ACT_DEN = 33, 128  # fraction of relu rows on ScalarE
SPLIT_ROWS = 6           # first rows emit half-width relu (start before full tT_h)
CH = 512                 # setup pipeline chunk


def _engine_map():
    """Per-i relu engine: 'A' (ScalarE), 'V' (VectorE)."""
    eng = []
    acc = 0
    for i in range(NI_LOC):
        acc += ACT_NUM
        if acc >= ACT_DEN:
            acc -= ACT_DEN
            eng.append("A")
        else:
            eng.append("V")
    return eng


def _build():
    import concourse.bacc as bacc
    import concourse.tile as tile
    import concourse.mybir as mybir

    F32 = mybir.dt.float32
    BF16 = mybir.dt.bfloat16
    AF = mybir.ActivationFunctionType
    ALU = mybir.AluOpType

    nc = bacc.Bacc("TRN2", target_bir_lowering=False, debug=False,
                   num_devices=N_CORES)

    # inputs split for parallel DMA issue from four engine queues
    txtA_d = nc.dram_tensor("txtA", [D, NT // 2], BF16,
                            kind="ExternalInput").ap()
    txtB_d = nc.dram_tensor("txtB", [D, NT // 2], BF16,
                            kind="ExternalInput").ap()
    big_d = nc.dram_tensor("big", [D, NI_LOC + 2 * H], BF16,
                           kind="ExternalInput").ap()
    wb128_d = nc.dram_tensor("wb128", [H, 3], F32, kind="ExternalInput").ap()
    pos_d = nc.dram_tensor("pos", [NI_LOC, NT], F32, kind="ExternalOutput").ap()
    # negO[p, 8i+jb] = negE-exp[i, 128jb+p]
    negO_d = nc.dram_tensor("negO", [H, 8 * NI_LOC], F32,
                            kind="ExternalOutput").ap()

    eng_map = _engine_map()

    with tile.TileContext(nc) as tc:
        with tc.tile_pool(name="const", bufs=1) as cpool, \
             tc.tile_pool(name="rp", bufs=R_BUFS) as rpool, \
             tc.tile_pool(name="op", bufs=1) as opool:

            # ---- load inputs from four queues in parallel; bf16 hashes
            #      (sign() is scale-invariant) ------------------------------
            txtT_raw = cpool.tile([D, NT], BF16)
            big = cpool.tile([D, NI_LOC + 2 * H], BF16)
            wb128 = cpool.tile([H, 3], F32)
            nc.gpsimd.dma_start(txtT_raw[:, 0:NT // 2], txtA_d[:])
            nc.scalar.dma_start(txtT_raw[:, NT // 2:NT], txtB_d[:])
            nc.sync.dma_start(big[:], big_d[:])
            nc.gpsimd.dma_start(wb128[:], wb128_d[:])
            imgT_raw = big[:, 0:NI_LOC]
            w1ti = big[:, NI_LOC:NI_LOC + H]
            w1tt = big[:, NI_LOC + H:NI_LOC + 2 * H]

            # ---- trigger the ACT exp table load early (after its dma issue) --
            warm = cpool.tile([1, 1], F32)
            nc.vector.memset(warm[:], 0.0)
            nc.scalar.activation(warm[:], warm[:], AF.Exp, bias=0.0, scale=1.0)
            b1c = wb128[:, 0:1]      # b1 - 0.125*W1.sum(1)  (folds sign=-1+m2)
            b2s = wb128[:, 1:2]      # 2*b2
            w2f = wb128[:, 2:3]

            # ---- m2 = (x>=0)*2 in {2,0} on VectorE (no ACT dependency) -------
            m2t = cpool.tile([D, NT], BF16)
            m2i = cpool.tile([D, NI_LOC], BF16)
            nc.vector.tensor_scalar(m2i[:], imgT_raw, 0.0, 2.0,
                                    op0=ALU.is_ge, op1=ALU.mult)
            for hh in range(0, NT, CH):
                nc.vector.tensor_scalar(m2t[:, hh:hh + CH],
                                        txtT_raw[:, hh:hh + CH], 0.0, 2.0,
                                        op0=ALU.is_ge, op1=ALU.mult)
            # sgn = m2 - 1 (exact +-1 bf16) for the pos path
            sgn_t = cpool.tile([D, NT], BF16)
            sgn_i = cpool.tile([D, NI_LOC], BF16)
            nc.vector.tensor_scalar(sgn_i[:], m2i[:], -1.0, None, op0=ALU.add)
            nc.vector.tensor_scalar(sgn_t[:], m2t[:], -1.0, None, op0=ALU.add)
            w2c = cpool.tile([H, 1], BF16)
            nc.vector.tensor_copy(w2c[:], w2f)

            # ---- h-transforms (tT_h kept resident in PSUM for ScalarE) -------
            tT_h = cpool.tile([H, NT], BF16)
            aT = cpool.tile([H, NI_LOC], F32)
            pos_sb = opool.tile([NI_LOC, NT], F32)

            with tc.tile_pool(name="ps_set", bufs=2, space="PSUM") as ps_s, \
                 tc.tile_pool(name="ps_a", bufs=1, space="PSUM") as ps_a:
                aps = ps_a.tile([H, NI_LOC], F32)
                nc.tensor.matmul(aps[:], lhsT=w1ti, rhs=m2i[:],
                                 start=True, stop=True)
                nc.vector.tensor_scalar(aT[:], aps[:], b1c, None, op0=ALU.add)

                for hh in range(0, NT, CH):
                    ps = ps_s.tile([H, CH], F32, tag="hps")
                    nc.tensor.matmul(ps[:], lhsT=w1tt,
                                     rhs=m2t[:, hh:hh + CH],
                                     start=True, stop=True)
                    nc.vector.tensor_copy(tT_h[:, hh:hh + CH], ps[:])

                for hh in range(0, NT, 512):
                    ps = ps_s.tile([NI_LOC, 512], F32, tag="pps")
                    nc.tensor.matmul(ps[:], lhsT=sgn_i[:],
                                     rhs=sgn_t[:, hh:hh + 512],
                                     start=True, stop=True)
                    nc.scalar.activation(pos_sb[:, hh:hh + 512], ps[:],
                                         AF.Exp, bias=0.0, scale=1.0 / 32.0)
            nc.sync.dma_start(pos_d[:], pos_sb[:])

            # ---- main pairwise loop: full-width pairs, per-32-i psum banks ---
            # psum tile t ([128, 256]) holds negE[i, j] at
            # [j%128, 8*(i%32) + j//128] for i in [32t, 32t+32)
            with tc.tile_pool(name="ps_m", bufs=1, space="PSUM") as ps_m:
                # full-bank tiles (cols 0:256 used) so eviction of tile t
                # never shares a bank with in-flight matmul writes
                psums = [ps_m.tile([H, 512], F32, tag=f"np{t}",
                                   name=f"negps{t}")
                         for t in range(4)]
                negsb = opool.tile([H, 8 * NI_LOC], F32)
                HW_ = NT // 2

                def emit_evict(t, c0=0, c1=256):
                    nc.scalar.activation(
                        negsb[:, 256 * t + c0:256 * t + c1],
                        psums[t][:, c0:c1],
                        AF.Exp, bias=b2s, scale=2.0)
                    nc.sync.dma_start(
                        negO_d[:, 256 * t + c0:256 * t + c1],
                        negsb[:, 256 * t + c0:256 * t + c1])

                pending = None
                for i in range(NI_LOC):
                    t = i // 32
                    colbase = 8 * (i % 32)
                    # defer the previous tile's eviction a dozen rows in so
                    # it never head-of-line-blocks ScalarE's relu stream
                    if pending is not None and i % 32 == 12:
                        emit_evict(pending)
                        pending = None
                    if i == 122:
                        # early half-evict of the last tile (i 96..111 done)
                        emit_evict(3, 0, 128)
                    if i < SPLIT_ROWS:
                        # two tiles so jb<4 matmuls only wait on the lo half
                        r_lo = rpool.tile([H, HW_], BF16, tag="rlo")
                        r_hi = rpool.tile([H, HW_], BF16, tag="rhi")
                        parts = [(r_lo, 0), (r_hi, HW_)]
                    else:
                        r = rpool.tile([H, NT], BF16, tag="r")
                        parts = [(r, 0)]
                    for rt, off in parts:
                        w = HW_ if i < SPLIT_ROWS else NT
                        if eng_map[i] == "A":
                            nc.scalar.activation(rt[:],
                                                 tT_h[:, off:off + w],
                                                 AF.Relu,
                                                 bias=aT[:, i:i + 1],
                                                 scale=1.0)
                        else:
                            nc.vector.tensor_scalar(rt[:],
                                                    tT_h[:, off:off + w],
                                                    aT[:, i:i + 1], 0.0,
                                                    op0=ALU.add,
                                                    op1=ALU.max)
                    for jb in range(NJB):
                        if i < SPLIT_ROWS:
                            rt, off = parts[jb // 4]
                            lhsT = rt[:, jb * H - off:jb * H - off + H]
                        else:
                            lhsT = parts[0][0][:, jb * H:(jb + 1) * H]
                        nc.tensor.matmul(
                            psums[t][:, colbase + jb:colbase + jb + 1],
                            lhsT=lhsT, rhs=w2c[:], start=True, stop=True)
                    if i % 32 == 31 and i != NI_LOC - 1:
                        pending = t
                emit_evict(3, 128, 256)

    nc.compile()
    return nc


def _get_compiled():
    global _compiled
    if _compiled is None:
        _compiled = _build()
    return _compiled


def run(inputs: dict, trace: bool = False):
    """Shard, run on 8 cores, gather. Returns (full_output, BassKernelResults)."""
    from concourse.bass_utils import run_bass_kernel_spmd

    nc = _get_compiled()

    imgs = np.asarray(inputs["images_hash"], dtype=np.float32)
    txts = np.asarray(inputs["texts_hash"], dtype=np.float32)
    W1 = np.asarray(inputs["W1"], dtype=np.float32)
    b1 = np.asarray(inputs["b1"], dtype=np.float32)
    W2 = np.asarray(inputs["W2"], dtype=np.float32)
    b2 = np.asarray(inputs["b2"], dtype=np.float32)
    task = int(np.asarray(inputs["task_is_i2t"]))

    bf16 = ml_dtypes.bfloat16
    txtT = np.ascontiguousarray(txts.T).astype(bf16)                # [64, 1024]
    wb64 = np.concatenate(
        [W1[:, :D].T * 0.125, W1[:, D:].T * 0.125], axis=1).astype(bf16)
    # fold sign = m2 - 1: b1_eff = b1 - 0.125 * W1.sum(axis=1)
    b1_eff = (b1 - 0.125 * W1.sum(axis=1)).astype(np.float32)
    wb128 = np.stack(
        [b1_eff, np.full(H, 2.0 * float(b2[0]), np.float32), W2[0]],
        axis=1).astype(np.float32)

    txtA = np.ascontiguousarray(txtT[:, :NT // 2])
    txtB = np.ascontiguousarray(txtT[:, NT // 2:])
    in_maps = []
    for c in range(N_CORES):
        sl = imgs[c * NI_LOC:(c + 1) * NI_LOC]
        imgT = np.ascontiguousarray(sl.T).astype(bf16)
        big = np.concatenate([imgT, wb64], axis=1)
        in_maps.append({"txtA": txtA, "txtB": txtB,
                        "big": big, "wb128": wb128})

    res = run_bass_kernel_spmd(nc, in_maps, list(range(N_CORES)), trace=trace)

    full = np.empty((NI * NT, 2), dtype=np.float32)
    pos = np.concatenate([res.results[c]["pos"] for c in range(N_CORES)], axis=0)
    # negO[p, 8i+jb] = negE-exp[i, 128jb+p]
    neg = np.concatenate(
        [res.results[c]["negO"].reshape(H, NI_LOC, NJB).transpose(1, 2, 0)
         .reshape(NI_LOC, NT) for c in range(N_CORES)], axis=0)
    full[:, 0] = (pos if task else pos.T).reshape(-1)
    full[:, 1] = neg.reshape(-1)
    return full, res


def kernel(**inputs) -> np.ndarray:
    out, _ = run(inputs, trace=False)
    return out
